# revision 15
# baseline (speedup 1.0000x reference)
"""Trainium2 Bass kernel for single-query gated cross-attention (DAttention).

Reference computation (per batch b, single query token at `pos`):
    q   = x[:, pos] @ Wq.T, scaled, split into 8 heads of 64
    kv  = context @ Wkv.T ; k, v = split(kv)
    dots = q @ k.T + attn_bias ; attn = softmax(mask(dots))
    out = (attn @ v) * sigmoid(x[:, pos] @ Wg.T + bg) @ Wo.T + bo

Algebraic optimization: with a single query token the full K/V projections
(the dominant 69 GFLOP) are unnecessary:
    dots[b,h,j] = sum_c context[b,j,c] * qk[b,h,c],   qk = (q_scaled @ Wk_h)
    attn-weighted V = (sum_j attn[b,h,j] * context[b,j,c]) @ Wv_h.T
The device computes dots, exp, and the attention-weighted context sum;
the tiny O(batch) folds (Wq/Wk, Wv, gating, Wo, softmax normalize) run
on host.

Device design (v2, HBM-bound at ~12.7 MB/core vs 16.8 for v1):
  * dots pass reads a TRANSPOSED fp8e4m3 copy of context (4.2 MB/core)
    as the matmul stationary operand, producing dots TOKEN-MAJOR
    [128 tok, heads] directly -- no PE transposes of the attention row
    needed (v1 spent ~18us of PE + DVE on transposing attn to token
    major). qk is split into fp8 hi+lo pairs so only the context's own
    fp8 rounding perturbs dots (~1% weight error; well within 2e-2).
  * exp on ACT engine with scale=1/256 folding the fp8 scaling away;
    bias (pre-scaled x256, mask folded as -1e30) added on the otherwise
    idle DVE. attn stays bf16.
  * weighted-sum pass reads NATURAL bf16 context (8.4 MB/core): lhsT =
    attn tile [128,8], rhs = ctx tile [128,512] accumulated over the 32
    token tiles into PSUM [8,512]. fp8 here would breach tolerance (the
    rounding error of a weighted mean does not average down).
  * denominators via a second tiny matmul per tile (rhs = ones [128,1])
    reusing the same stationary; normalization happens on host.
  * DMA order: all ctxT (fp8) first -- dots fill the PE while the 2x
    larger ctxn stream follows; wsum trails the ctxn stream group by
    group, leaving only a ~2us tail after the last byte.

Sharding: data-parallel over batch (16 batches / 8 cores = 2 per core).
No collectives; host gathers per-core [2, 8, 512] sums + denominators.
"""

import numpy as np
import ml_dtypes

import concourse.bass as bass
import concourse.bacc as bacc
import concourse.tile as tile
import concourse.mybir as mybir
from concourse.bass_utils import run_bass_kernel_spmd

BF16 = mybir.dt.bfloat16
FP8 = mybir.dt.float8e4
F32 = mybir.dt.float32
NP_BF16 = ml_dtypes.bfloat16
# TRN2 supports the IEEE-ish E4M3 (max +-240), not e4m3fn (NCC_EVRF051)
NP_FP8 = ml_dtypes.float8_e4m3

N_CORES = 8
B = 16
N = 4096
DIM = 512
HEADS = 8
DIM_HEAD = 64
INNER = HEADS * DIM_HEAD
SCALE = DIM_HEAD ** -0.5
BPC = B // N_CORES          # batches per core (2)
KC = DIM // 128             # contraction chunks (4)
NT = N // 128               # token tiles of 128 (32)
GT = 8                      # token tiles per exp group
NG = NT // GT               # groups per batch (4)
QS = 256.0                  # fp8 scaling of qk (and bias)


def _build_nc():
    """Build + compile the SPMD single-core program (identical on all cores)."""
    nc = bacc.Bacc("TRN2", target_bir_lowering=False, debug=False,
                   num_devices=N_CORES)

    # DRAM I/O (per-core shapes), all partition-major so every DMA is a
    # contiguous per-partition run (cheap HWDGE descriptors)
    ctxT_d = nc.dram_tensor("ctxT", [BPC, 128, KC, N], FP8,
                            kind="ExternalInput")
    # ctxn: natural context, bf16, token-quarters
    ctxn_d = nc.dram_tensor("ctxn", [BPC, 4, 128, NT // 4, DIM], BF16,
                            kind="ExternalInput")
    # qk2: folded query, fp8 hi/lo pairs, x256
    qk2_d = nc.dram_tensor("qk2", [128, KC, BPC, 16], FP8,
                           kind="ExternalInput")
    # bias: token-major bias x256 with mask folded, bf16
    bias_d = nc.dram_tensor("biasT", [BPC, 128, NT, HEADS], BF16,
                            kind="ExternalInput")
    ones_d = nc.dram_tensor("ones1", [128, 1], BF16, kind="ExternalInput")
    # acc cols 0..511, denominator col 512; host splits + normalizes
    out_d = nc.dram_tensor("out", [BPC, HEADS, DIM + 1], F32,
                           kind="ExternalOutput")

    with tile.TileContext(nc) as tc:
        with (
            tc.tile_pool(name="const", bufs=1) as const_pool,
            tc.tile_pool(name="ctxT", bufs=1) as ctxT_pool,
            tc.tile_pool(name="ctxn", bufs=1) as ctxn_pool,
            tc.tile_pool(name="attn", bufs=1) as attn_pool,
            tc.tile_pool(name="work", bufs=2) as work_pool,
            tc.tile_pool(name="pd", bufs=2, space="PSUM") as pd_pool,
            tc.tile_pool(name="pacc", bufs=1, space="PSUM") as pacc_pool,
        ):
            # ---- big context loads first (SP HWDGE ring) ----
            # ctxT (fp8) leads: dots depend only on it; ctxn follows in
            # consumption order so wsum trails the stream tightly.
            ctxT_sb = []
            for b in range(BPC):
                t = ctxT_pool.tile([128, KC, N], FP8,
                                   tag=f"ctxT{b}", name=f"ctxT{b}")
                nc.sync.dma_start(out=t[:], in_=ctxT_d[b])
                ctxT_sb.append(t)
            ctxn_sb = [[None] * 4 for _ in range(BPC)]
            for b in range(BPC):
                for qt in range(4):
                    t = ctxn_pool.tile([128, NT // 4, DIM], BF16,
                                       tag=f"ctxn{b}{qt}", name=f"ctxn{b}{qt}")
                    nc.sync.dma_start(out=t[:], in_=ctxn_d[b, qt])
                    ctxn_sb[b][qt] = t

            # ---- small inputs (ACT HWDGE ring, concurrent with SP ring) ----
            qk2_sb = const_pool.tile([128, KC, BPC, 16], FP8, tag="qk2")
            nc.scalar.dma_start(out=qk2_sb[:], in_=qk2_d[:])
            ones_sb = const_pool.tile([128, 1], BF16, tag="ones")
            nc.scalar.dma_start(out=ones_sb[:], in_=ones_d[:])
            bias_sb = []
            for b in range(BPC):
                t = const_pool.tile([128, NT, HEADS], BF16, tag=f"bias{b}",
                                    name=f"bias{b}")
                nc.scalar.dma_start(out=t[:], in_=bias_d[b])
                bias_sb.append(t)

            # persistent attention weights, token-major [128, NT, 8] bf16
            attn = [attn_pool.tile([128, NT, HEADS], BF16, tag=f"attn{b}",
                                   name=f"attn{b}") for b in range(BPC)]

            pacc = [pacc_pool.tile([HEADS, DIM], F32, tag=f"pa{b}",
                                   name=f"pa{b}") for b in range(BPC)]
            pden = pacc_pool.tile([HEADS, BPC], F32, tag="pden")

            # ---- phase 1: dots + exp for both batches (needs only ctxT,
            # the first 4.2 MB of the DMA stream) ----
            for b in range(BPC):
                for g in range(NG):
                    pd = pd_pool.tile([128, GT * 16], F32, tag="pd")
                    for ti in range(GT):
                        jt = g * GT + ti            # global token tile
                        for k in range(KC):
                            nc.tensor.matmul(
                                pd[:, ti * 16:(ti + 1) * 16],
                                lhsT=ctxT_sb[b][:, k,
                                               jt * 128:(jt + 1) * 128],
                                rhs=qk2_sb[:, k, b, :],
                                start=(k == 0),
                                stop=(k == KC - 1),
                            )
                    # combine hi+lo qk halves and add bias (idle DVE).
                    # Each op reads PSUM at most once (NCC_IBVF027).
                    pdv = pd.rearrange("p (t two h) -> p t two h", two=2,
                                       h=HEADS)
                    dsum = work_pool.tile([128, GT, HEADS], F32, tag="dsum")
                    nc.vector.tensor_tensor(
                        out=dsum[:], in0=pdv[:, :, 0, :],
                        in1=bias_sb[b][:, g * GT:(g + 1) * GT, :],
                        op=mybir.AluOpType.add)
                    nc.vector.tensor_tensor(
                        out=dsum[:], in0=pdv[:, :, 1, :], in1=dsum[:],
                        op=mybir.AluOpType.add)
                    # exp (scale folds away the x256 fp8 scaling) -> bf16
                    nc.scalar.activation(
                        attn[b][:, g * GT:(g + 1) * GT, :], dsum[:],
                        mybir.ActivationFunctionType.Exp, scale=1.0 / QS)
            # ---- PE warmup spin: ~4us of dummy matmuls on already-loaded
            # data. The PE idles waiting for the first ctxn quarter here;
            # without this the HAM clock gate re-throttles to 1.2 GHz and a
            # cold wsum (427ns/MM) is SLOWER than the 358GB/s ctxn stream,
            # trailing it by ~5us. Warm (213ns/MM) it hides completely.
            pwarm = pacc_pool.tile([16, DIM], F32, tag="pwarm")
            for _ in range(10):
                nc.tensor.matmul(
                    pwarm[:],
                    lhsT=qk2_sb[:, 0, 0, :],
                    rhs=ctxT_sb[0][:, 0, 0:DIM],
                    start=True, stop=True,
                )
            # ---- phase 2: attention-weighted context sum + denominators,
            # trailing the ctxn stream quarter by quarter ----
            for b in range(BPC):
                for jt in range(NT):
                    w = attn[b][:, jt, :]
                    nc.tensor.matmul(
                        pacc[b][:],
                        lhsT=w,
                        rhs=ctxn_sb[b][jt // (NT // 4)][:, jt % (NT // 4)],
                        start=(jt == 0),
                        stop=(jt == NT - 1),
                    )
                    nc.tensor.matmul(
                        pden[:, b:b + 1],
                        lhsT=w,
                        rhs=ones_sb[:],
                        start=(jt == 0),
                        stop=(jt == NT - 1),
                    )
                # ship unnormalized sums + denominator in one DMA; host divides
                outt = work_pool.tile([HEADS, DIM + 1], F32, tag="outt")
                nc.vector.tensor_copy(outt[:, :DIM], pacc[b][:])
                nc.vector.tensor_copy(outt[:, DIM:DIM + 1], pden[:, b:b + 1])
                nc.scalar.dma_start(out=out_d[b], in_=outt[:])

    nc.compile()
    return nc


_NC_CACHE = None


def _get_nc():
    global _NC_CACHE
    if _NC_CACHE is None:
        _NC_CACHE = _build_nc()
    return _NC_CACHE


def _host_prep(x, context, attn_bias, Wq, Wkv, Wg, bg, mask, context_mask, pos):
    """Fold the query-side projections and build per-core device inputs."""
    pos = int(pos)
    qx = np.asarray(x[:, pos, :], dtype=np.float32)              # [B, DIM]
    Wq = np.asarray(Wq, np.float32)
    Wkv = np.asarray(Wkv, np.float32)
    q = (qx @ Wq.T).reshape(B, HEADS, DIM_HEAD) * SCALE          # [B, 8, 64]
    Wk = Wkv[:INNER].reshape(HEADS, DIM_HEAD, DIM)               # [8, 64, DIM]
    qk = np.einsum("bhd,hdc->bhc", q, Wk) * QS                   # [B, 8, DIM]

    # hi/lo fp8 split of qk: residual rounding error ~0.1%
    qk_hi = qk.astype(NP_FP8)
    qk_lo = (qk - qk_hi.astype(np.float32)).astype(NP_FP8)
    # partition-major [128, KC, B, 16]
    qk2 = np.zeros((DIM, B, 16), dtype=NP_FP8)
    qk2[:, :, 0:8] = qk_hi.transpose(2, 0, 1)
    qk2[:, :, 8:16] = qk_lo.transpose(2, 0, 1)
    qk2 = np.ascontiguousarray(
        qk2.reshape(KC, 128, B, 16).transpose(1, 0, 2, 3))

    # bias x256 with masking folded in (-1e30 -> exp underflows to 0),
    # token-major [B, 128, NT, H]
    full_mask = (np.asarray(mask, bool).reshape(B, 1, 1)
                 & np.asarray(context_mask, bool).reshape(B, 1, N))
    biasf = np.where(full_mask,
                     np.asarray(attn_bias, np.float32).reshape(B, HEADS, N),
                     -1e30) * QS
    biasT = np.ascontiguousarray(
        biasf.reshape(B, HEADS, NT, 128).transpose(0, 3, 2, 1)).astype(NP_BF16)

    ctxf = np.asarray(context, np.float32)                       # [B, N, DIM]
    ones1 = np.ones((128, 1), dtype=NP_BF16)
    in_maps = []
    for c in range(N_CORES):
        bs = slice(c * BPC, (c + 1) * BPC)
        ctx_c = ctxf[bs]
        # [BPC, 128, KC, N] fp8 transposed, partition-major
        ctxT = np.ascontiguousarray(
            ctx_c.transpose(0, 2, 1).reshape(BPC, KC, 128, N)
            .transpose(0, 2, 1, 3)).astype(NP_FP8)
        # [BPC, 4, 128, NT//4, DIM] bf16 natural
        ctxn = np.ascontiguousarray(
            ctx_c.reshape(BPC, 4, NT // 4, 128, DIM).transpose(0, 1, 3, 2, 4)
        ).astype(NP_BF16)
        in_maps.append({
            "ctxT": ctxT,
            "ctxn": ctxn,
            "qk2": np.ascontiguousarray(qk2[:, :, bs]),
            "biasT": np.ascontiguousarray(biasT[bs]),
            "ones1": ones1,
        })
    return in_maps


def _host_epilogue(acc, den, x, Wkv, Wo, bo, Wg, bg, pos):
    """acc[b,h,c]/den -> out[b,1,dim] via the Wv fold, gating and Wo."""
    pos = int(pos)
    qx = np.asarray(x[:, pos, :], dtype=np.float32)
    accn = acc / den.reshape(B, HEADS, 1)
    Wv = np.asarray(Wkv, np.float32)[INNER:].reshape(HEADS, DIM_HEAD, DIM)
    out_v = np.einsum("bhc,hdc->bhd", accn, Wv).reshape(B, INNER)
    gates = qx @ np.asarray(Wg, np.float32).T + np.asarray(bg, np.float32)
    inner = out_v * (1.0 / (1.0 + np.exp(-gates)))
    out = inner @ np.asarray(Wo, np.float32).T + np.asarray(bo, np.float32)
    return out.reshape(B, 1, DIM).astype(np.float32)


def run_device(in_maps, trace=False):
    nc = _get_nc()
    return run_bass_kernel_spmd(nc, in_maps, list(range(N_CORES)), trace=trace)


def kernel(x, context, attn_bias, Wq, Wkv, Wo, bo, Wg, bg, mask, context_mask,
           pos, _trace=False, _results=None):
    in_maps = _host_prep(x, context, attn_bias, Wq, Wkv, Wg, bg,
                         mask, context_mask, pos)
    res = run_device(in_maps, trace=_trace)
    if _results is not None:
        _results.append(res)
    out = np.concatenate([res.results[c]["out"] for c in range(N_CORES)],
                         axis=0).astype(np.float32)
    return _host_epilogue(out[:, :, :DIM], out[:, :, DIM], x, Wkv, Wo, bo,
                          Wg, bg, pos)


# revision 19
# speedup vs baseline: 1.0567x; 1.0567x over previous
"""Trainium2 Bass kernel for single-query gated cross-attention (DAttention).

Reference computation (per batch b, single query token at `pos`):
    q   = x[:, pos] @ Wq.T, scaled, split into 8 heads of 64
    kv  = context @ Wkv.T ; k, v = split(kv)
    dots = q @ k.T + attn_bias ; attn = softmax(mask(dots))
    out = (attn @ v) * sigmoid(x[:, pos] @ Wg.T + bg) @ Wo.T + bo

Algebraic optimization: with a single query token the full K/V projections
(the dominant 69 GFLOP) are unnecessary:
    dots[b,h,j] = sum_c context[b,j,c] * qk[b,h,c],   qk = (q_scaled @ Wk_h)
    attn-weighted V = (sum_j attn[b,h,j] * context[b,j,c]) @ Wv_h.T
The device computes dots, exp, and the attention-weighted context sum;
the tiny O(batch) folds (Wq/Wk, Wv, gating, Wo, softmax normalize) run
on host.

Device design (v2, HBM-bound at ~12.7 MB/core vs 16.8 for v1):
  * dots pass reads a TRANSPOSED fp8e4m3 copy of context (4.2 MB/core)
    as the matmul stationary operand, producing dots TOKEN-MAJOR
    [128 tok, heads] directly -- no PE transposes of the attention row
    needed (v1 spent ~18us of PE + DVE on transposing attn to token
    major). qk is split into fp8 hi+lo pairs so only the context's own
    fp8 rounding perturbs dots (~1% weight error; well within 2e-2).
  * exp on ACT engine with scale=1/256 folding the fp8 scaling away;
    bias (pre-scaled x256, mask folded as -1e30) added on the otherwise
    idle DVE. attn stays bf16.
  * weighted-sum pass reads NATURAL bf16 context (8.4 MB/core): lhsT =
    attn tile [128,8], rhs = ctx tile [128,512] accumulated over the 32
    token tiles into PSUM [8,512]. fp8 here would breach tolerance (the
    rounding error of a weighted mean does not average down).
  * denominators via a second tiny matmul per tile (rhs = ones [128,1])
    reusing the same stationary; normalization happens on host.
  * DMA order: all ctxT (fp8) first -- dots fill the PE while the 2x
    larger ctxn stream follows; wsum trails the ctxn stream group by
    group, leaving only a ~2us tail after the last byte.

Sharding: data-parallel over batch (16 batches / 8 cores = 2 per core).
No collectives; host gathers per-core [2, 8, 512] sums + denominators.
"""

import numpy as np
import ml_dtypes

import concourse.bass as bass
import concourse.bacc as bacc
import concourse.tile as tile
import concourse.mybir as mybir
from concourse.bass_utils import run_bass_kernel_spmd

BF16 = mybir.dt.bfloat16
FP8 = mybir.dt.float8e4
F32 = mybir.dt.float32
NP_BF16 = ml_dtypes.bfloat16
# TRN2 supports the IEEE-ish E4M3 (max +-240), not e4m3fn (NCC_EVRF051)
NP_FP8 = ml_dtypes.float8_e4m3

N_CORES = 8
B = 16
N = 4096
DIM = 512
HEADS = 8
DIM_HEAD = 64
INNER = HEADS * DIM_HEAD
SCALE = DIM_HEAD ** -0.5
BPC = B // N_CORES          # batches per core (2)
KC = DIM // 128             # contraction chunks (4)
NT = N // 128               # token tiles of 128 (32)
GT = 8                      # token tiles per exp group
NG = NT // GT               # groups per batch (4)
QS = 256.0                  # fp8 scaling of qk (and bias)


def _build_nc():
    """Build + compile the SPMD single-core program (identical on all cores)."""
    nc = bacc.Bacc("TRN2", target_bir_lowering=False, debug=False,
                   num_devices=N_CORES)

    # DRAM I/O (per-core shapes), all partition-major so every DMA is a
    # contiguous per-partition run (cheap HWDGE descriptors)
    ctxT_d = nc.dram_tensor("ctxT", [BPC, 2, 128, KC, N // 2], FP8,
                            kind="ExternalInput")
    # ctxn: natural context, bf16, token-quarters
    ctxn_d = nc.dram_tensor("ctxn", [BPC, 4, 128, NT // 4, DIM], BF16,
                            kind="ExternalInput")
    # qk2: folded query, fp8 hi/lo pairs, x256
    qk2_d = nc.dram_tensor("qk2", [128, KC, BPC, 16], FP8,
                           kind="ExternalInput")
    # bias: token-major bias x256 with mask folded, bf16
    bias_d = nc.dram_tensor("biasT", [BPC, 128, NT, HEADS], BF16,
                            kind="ExternalInput")
    ones_d = nc.dram_tensor("ones1", [128, 1], BF16, kind="ExternalInput")
    # acc cols 0..511, denominator col 512; host splits + normalizes
    out_d = nc.dram_tensor("out", [BPC, HEADS, DIM + 1], F32,
                           kind="ExternalOutput")

    with tile.TileContext(nc) as tc:
        with (
            tc.tile_pool(name="const", bufs=1) as const_pool,
            tc.tile_pool(name="ctxT", bufs=1) as ctxT_pool,
            tc.tile_pool(name="ctxn", bufs=1) as ctxn_pool,
            tc.tile_pool(name="attn", bufs=1) as attn_pool,
            tc.tile_pool(name="work", bufs=2) as work_pool,
            tc.tile_pool(name="pd", bufs=2, space="PSUM") as pd_pool,
            tc.tile_pool(name="pacc", bufs=1, space="PSUM") as pacc_pool,
        ):
            # ---- big context loads first (SP HWDGE ring) ----
            # ctxT (fp8) leads: dots depend only on it; ctxn follows in
            # consumption order so wsum trails the stream tightly.
            # DMA issue order tracks the PE consumption order below so the
            # PE never idles past the ~3.4us HAM re-throttle window.
            ctxT_sb = [[None] * 2 for _ in range(BPC)]
            ctxn_sb = [[None] * 4 for _ in range(BPC)]

            def load_T(b, hf):
                t = ctxT_pool.tile([128, KC, N // 2], FP8,
                                   tag=f"ctxT{b}{hf}", name=f"ctxT{b}{hf}")
                nc.sync.dma_start(out=t[:], in_=ctxT_d[b, hf])
                ctxT_sb[b][hf] = t

            def load_N(b, qt):
                t = ctxn_pool.tile([128, NT // 4, DIM], BF16,
                                   tag=f"ctxn{b}{qt}", name=f"ctxn{b}{qt}")
                nc.sync.dma_start(out=t[:], in_=ctxn_d[b, qt])
                ctxn_sb[b][qt] = t

            load_T(0, 0); load_T(0, 1)
            load_N(0, 0); load_N(0, 1)
            load_T(1, 0)
            load_N(0, 2)
            load_T(1, 1)
            load_N(0, 3)
            for qt in range(4):
                load_N(1, qt)

            # ---- small inputs (ACT HWDGE ring, concurrent with SP ring) ----
            qk2_sb = const_pool.tile([128, KC, BPC, 16], FP8, tag="qk2")
            nc.scalar.dma_start(out=qk2_sb[:], in_=qk2_d[:])
            ones_sb = const_pool.tile([128, 1], BF16, tag="ones")
            nc.scalar.dma_start(out=ones_sb[:], in_=ones_d[:])
            bias_sb = []
            for b in range(BPC):
                t = const_pool.tile([128, NT, HEADS], BF16, tag=f"bias{b}",
                                    name=f"bias{b}")
                nc.scalar.dma_start(out=t[:], in_=bias_d[b])
                bias_sb.append(t)

            # persistent attention weights, token-major [128, NT, 8] bf16
            attn = [attn_pool.tile([128, NT, HEADS], BF16, tag=f"attn{b}",
                                   name=f"attn{b}") for b in range(BPC)]

            pacc = [pacc_pool.tile([HEADS, DIM], F32, tag=f"pa{b}",
                                   name=f"pa{b}") for b in range(BPC)]
            pden = pacc_pool.tile([HEADS, BPC], F32, tag="pden")

            # ---- compute building blocks ----
            def dots_group(b, g):
                pd = pd_pool.tile([128, GT * 16], F32, tag="pd")
                for ti in range(GT):
                    jt = g * GT + ti                # global token tile
                    hf, loc = divmod(jt, NT // 2)
                    for k in range(KC):
                        nc.tensor.matmul(
                            pd[:, ti * 16:(ti + 1) * 16],
                            lhsT=ctxT_sb[b][hf][:, k,
                                                loc * 128:(loc + 1) * 128],
                            rhs=qk2_sb[:, k, b, :],
                            start=(k == 0),
                            stop=(k == KC - 1),
                        )
                # combine hi+lo qk halves and add bias (idle DVE).
                # Each op reads PSUM at most once (NCC_IBVF027).
                pdv = pd.rearrange("p (t two h) -> p t two h", two=2, h=HEADS)
                dsum = work_pool.tile([128, GT, HEADS], F32, tag="dsum")
                nc.vector.tensor_tensor(
                    out=dsum[:], in0=pdv[:, :, 0, :],
                    in1=bias_sb[b][:, g * GT:(g + 1) * GT, :],
                    op=mybir.AluOpType.add)
                nc.vector.tensor_tensor(
                    out=dsum[:], in0=pdv[:, :, 1, :], in1=dsum[:],
                    op=mybir.AluOpType.add)
                # exp (scale folds away the x256 fp8 scaling) -> bf16
                nc.scalar.activation(
                    attn[b][:, g * GT:(g + 1) * GT, :], dsum[:],
                    mybir.ActivationFunctionType.Exp, scale=1.0 / QS)

            def wsum_quarter(b, q):
                for jt in range(q * GT, (q + 1) * GT):
                    w = attn[b][:, jt, :]
                    nc.tensor.matmul(
                        pacc[b][:],
                        lhsT=w,
                        rhs=ctxn_sb[b][jt // (NT // 4)][:, jt % (NT // 4)],
                        start=(jt == 0),
                        stop=(jt == NT - 1),
                    )
                    nc.tensor.matmul(
                        pden[:, b:b + 1],
                        lhsT=w,
                        rhs=ones_sb[:],
                        start=(jt == 0),
                        stop=(jt == NT - 1),
                    )

            def ship(b):
                # unnormalized sums + denominator in one DMA; host divides
                outt = work_pool.tile([HEADS, DIM + 1], F32, tag="outt")
                nc.vector.tensor_copy(outt[:, :DIM], pacc[b][:])
                nc.vector.tensor_copy(outt[:, DIM:DIM + 1], pden[:, b:b + 1])
                nc.scalar.dma_start(out=out_d[b], in_=outt[:])

            # ---- PE program, interleaved to track the DMA stream with no
            # idle gap past the ~3.4us HAM re-throttle window ----
            for g in range(NG):
                dots_group(0, g)
            # small spin bridging the dots->wsum handoff (keeps HAM warm
            # while the first ctxn quarter's completion semaphore lands)
            pwarm = pacc_pool.tile([16, DIM], F32, tag="pwarm")
            for _ in range(6):
                nc.tensor.matmul(
                    pwarm[:],
                    lhsT=qk2_sb[:, 0, 0, :],
                    rhs=ctxT_sb[0][0][:, 0, 0:DIM],
                    start=True, stop=True,
                )
            wsum_quarter(0, 0)
            wsum_quarter(0, 1)
            dots_group(1, 0)
            dots_group(1, 1)
            wsum_quarter(0, 2)
            dots_group(1, 2)
            dots_group(1, 3)
            wsum_quarter(0, 3)
            ship(0)
            for q in range(4):
                wsum_quarter(1, q)
            ship(1)

    nc.compile()
    return nc


_NC_CACHE = None


def _get_nc():
    global _NC_CACHE
    if _NC_CACHE is None:
        _NC_CACHE = _build_nc()
    return _NC_CACHE


def _host_prep(x, context, attn_bias, Wq, Wkv, Wg, bg, mask, context_mask, pos):
    """Fold the query-side projections and build per-core device inputs."""
    pos = int(pos)
    qx = np.asarray(x[:, pos, :], dtype=np.float32)              # [B, DIM]
    Wq = np.asarray(Wq, np.float32)
    Wkv = np.asarray(Wkv, np.float32)
    q = (qx @ Wq.T).reshape(B, HEADS, DIM_HEAD) * SCALE          # [B, 8, 64]
    Wk = Wkv[:INNER].reshape(HEADS, DIM_HEAD, DIM)               # [8, 64, DIM]
    qk = np.einsum("bhd,hdc->bhc", q, Wk) * QS                   # [B, 8, DIM]

    # hi/lo fp8 split of qk: residual rounding error ~0.1%
    qk_hi = qk.astype(NP_FP8)
    qk_lo = (qk - qk_hi.astype(np.float32)).astype(NP_FP8)
    # partition-major [128, KC, B, 16]
    qk2 = np.zeros((DIM, B, 16), dtype=NP_FP8)
    qk2[:, :, 0:8] = qk_hi.transpose(2, 0, 1)
    qk2[:, :, 8:16] = qk_lo.transpose(2, 0, 1)
    qk2 = np.ascontiguousarray(
        qk2.reshape(KC, 128, B, 16).transpose(1, 0, 2, 3))

    # bias x256 with masking folded in (-1e30 -> exp underflows to 0),
    # token-major [B, 128, NT, H]
    full_mask = (np.asarray(mask, bool).reshape(B, 1, 1)
                 & np.asarray(context_mask, bool).reshape(B, 1, N))
    biasf = np.where(full_mask,
                     np.asarray(attn_bias, np.float32).reshape(B, HEADS, N),
                     -1e30) * QS
    biasT = np.ascontiguousarray(
        biasf.reshape(B, HEADS, NT, 128).transpose(0, 3, 2, 1)).astype(NP_BF16)

    ctxf = np.asarray(context, np.float32)                       # [B, N, DIM]
    ones1 = np.ones((128, 1), dtype=NP_BF16)
    in_maps = []
    for c in range(N_CORES):
        bs = slice(c * BPC, (c + 1) * BPC)
        ctx_c = ctxf[bs]
        # [BPC, 2, 128, KC, N//2] fp8 transposed, partition-major token halves
        ctxT = np.ascontiguousarray(
            ctx_c.transpose(0, 2, 1).reshape(BPC, KC, 128, 2, N // 2)
            .transpose(0, 3, 2, 1, 4)).astype(NP_FP8)
        # [BPC, 4, 128, NT//4, DIM] bf16 natural
        ctxn = np.ascontiguousarray(
            ctx_c.reshape(BPC, 4, NT // 4, 128, DIM).transpose(0, 1, 3, 2, 4)
        ).astype(NP_BF16)
        in_maps.append({
            "ctxT": ctxT,
            "ctxn": ctxn,
            "qk2": np.ascontiguousarray(qk2[:, :, bs]),
            "biasT": np.ascontiguousarray(biasT[bs]),
            "ones1": ones1,
        })
    return in_maps


def _host_epilogue(acc, den, x, Wkv, Wo, bo, Wg, bg, pos):
    """acc[b,h,c]/den -> out[b,1,dim] via the Wv fold, gating and Wo."""
    pos = int(pos)
    qx = np.asarray(x[:, pos, :], dtype=np.float32)
    accn = acc / den.reshape(B, HEADS, 1)
    Wv = np.asarray(Wkv, np.float32)[INNER:].reshape(HEADS, DIM_HEAD, DIM)
    out_v = np.einsum("bhc,hdc->bhd", accn, Wv).reshape(B, INNER)
    gates = qx @ np.asarray(Wg, np.float32).T + np.asarray(bg, np.float32)
    inner = out_v * (1.0 / (1.0 + np.exp(-gates)))
    out = inner @ np.asarray(Wo, np.float32).T + np.asarray(bo, np.float32)
    return out.reshape(B, 1, DIM).astype(np.float32)


def run_device(in_maps, trace=False):
    nc = _get_nc()
    return run_bass_kernel_spmd(nc, in_maps, list(range(N_CORES)), trace=trace)


def kernel(x, context, attn_bias, Wq, Wkv, Wo, bo, Wg, bg, mask, context_mask,
           pos, _trace=False, _results=None):
    in_maps = _host_prep(x, context, attn_bias, Wq, Wkv, Wg, bg,
                         mask, context_mask, pos)
    res = run_device(in_maps, trace=_trace)
    if _results is not None:
        _results.append(res)
    out = np.concatenate([res.results[c]["out"] for c in range(N_CORES)],
                         axis=0).astype(np.float32)
    return _host_epilogue(out[:, :, :DIM], out[:, :, DIM], x, Wkv, Wo, bo,
                          Wg, bg, pos)
